# revision 1
# baseline (speedup 1.0000x reference)
"""Causal GQA self-attention (B=2, S=2048, D=2048, 32 Q heads / 8 KV heads,
head_dim 64, RoPE) on 8 Trainium2 NeuronCores.

Sharding: data-parallel over batch (2) x tensor-parallel over heads (4).
Core c handles batch c//4 and head group c%4 (8 Q heads, 2 KV heads).
wq/wk/wv column-sharded, wo row-sharded; the 4 partial outputs per batch
are summed on the host at gather time (the "all-reduce").

Device kernel (per core), everything in transposed [dims, seq] layout:
  QT = wq_g.T @ x.T   (via lhsT=wq chunks, rhs=xT tiles)   [512, 2048]
  KT = wk_g.T @ x.T                                        [128, 2048]
  V  = x @ wv_g       (via lhsT=xT chunks, rhs=wv)         [2048, 128]
  RoPE: host de-interleaves wq/wk cols to [real(32)|imag(32)] per head;
    rot(t) = t*C4 + swap(t)*S4 where swap exchanges r/i blocks (done via
    SBUF->SBUF DMA across partitions) and S4 carries [-sin|+sin] blocks.
  scoresT[k,q] = KT_h.T @ QT_h  per head, causal tiles only
  expT = exp(scoresT)  (no max subtraction; scores are O(6))
  mask straddling diagonal tiles with precomputed 0/1 masks
  PV: lhsT = [V|1] (kv0) / [1|V] (kv1) so the denominator row lands
    adjacent to the value rows at the half-aligned partition offset
  normalize folded into PSUM->SBUF eviction (PE-broadcast recip row);
    all APs share base partition 64*half to satisfy walrus
  out_partial = attnT.T @ wo_g  (attnT is directly the lhsT)

Matmuls run as float32r (fp32 bits, ~19-bit effective mantissa, full PE
rate at N>=256) -- measured ~2e-4 rel err vs fp64 at K=128.
"""

import sys

if "/opt/trn_rl_repo" not in sys.path:
    sys.path.insert(0, "/opt/trn_rl_repo")

import numpy as np

import concourse.bass as bass
import concourse.tile as tile
from concourse import bacc, mybir
from concourse.bass_utils import run_bass_kernel_spmd

B = 2
S = 2048
D = 2048
N_HEAD = 32
N_KV = 8
HD = 64
GROUPS = 4
HQ = N_HEAD // GROUPS
HK = N_KV // GROUPS
QD = HQ * HD
KD = HK * HD
P = 128
SB = 512
NB = S // SB
DC = D // P
QC = QD // P

F32 = mybir.dt.float32
F32R = mybir.dt.float32r
USE_F32R = True
RDT = F32R if USE_F32R else F32


DEBUG_DUMPS = False


def build_kernel():
    nc = bacc.Bacc("TRN2", target_bir_lowering=False, debug=False,
                   num_devices=8)

    xT = nc.dram_tensor("xT", (D, S), F32, kind="ExternalInput").ap()
    wq = nc.dram_tensor("wq", (D, QD), F32, kind="ExternalInput").ap()
    wkv = nc.dram_tensor("wkv", (D, KD + KD), F32, kind="ExternalInput").ap()
    wo = nc.dram_tensor("wo", (QD, D), F32, kind="ExternalInput").ap()
    c4 = nc.dram_tensor("c4", (P, S), F32, kind="ExternalInput").ap()
    s4 = nc.dram_tensor("s4", (P, S), F32, kind="ExternalInput").ap()
    cmask = nc.dram_tensor("cmask", (4, P, SB), F32, kind="ExternalInput").ap()
    eye = nc.dram_tensor("eye", (HD, P), F32, kind="ExternalInput").ap()
    eye128 = nc.dram_tensor("eye128", (P, P), F32, kind="ExternalInput").ap()
    outp = nc.dram_tensor("outp", (S, D), F32, kind="ExternalOutput").ap()
    dumps = None
    if DEBUG_DUMPS:
        dumps = {
            "qtd": nc.dram_tensor("qtd", (QD, S), F32,
                                  kind="ExternalOutput").ap(),
            "ktd": nc.dram_tensor("ktd", (P, S), F32,
                                  kind="ExternalOutput").ap(),
            "vd": nc.dram_tensor("vd", (DC * P, 2 * (HD + 1)), F32,
                                 kind="ExternalOutput").ap(),
            "ad": nc.dram_tensor("ad", (QD, S), F32,
                                 kind="ExternalOutput").ap(),
        }

    with tile.TileContext(nc) as tc, \
         nc.allow_low_precision(reason="fp32r matmul operands"):
        _body(nc, tc, xT, wq, wkv, wo, c4, s4, cmask, eye, eye128, outp, dumps)

    nc.compile()
    return nc


def _body(nc, tc, xT, wq, wkv, wo, c4, s4, cmask, eye, eye128, outp, dumps=None):
    from contextlib import ExitStack

    ctx = ExitStack()
    with ctx:
        # ---- persistent SBUF pools ----
        qt_pool = ctx.enter_context(tc.tile_pool(name="qt", bufs=QC))
        kt_pool = ctx.enter_context(tc.tile_pool(name="kt", bufs=1))
        vaug_pool = ctx.enter_context(tc.tile_pool(name="vaug", bufs=DC))
        attn_pool = ctx.enter_context(tc.tile_pool(name="attnT", bufs=QC))
        singles = ctx.enter_context(tc.tile_pool(name="singles", bufs=1))

        cm_sb = []
        for m in range(4):
            t = singles.tile([P, SB], F32, tag=f"cm{m}", name=f"cm{m}")
            nc.sync.dma_start(t[:], cmask[m])
            cm_sb.append(t)
        # memset cannot produce f32r: memset F32 then ACT-copy to f32r
        ones_f32 = singles.tile([P, HD], F32, tag="ones_f32")
        nc.vector.memset(ones_f32[:], 1.0)
        ones_sb = singles.tile([P, HD], RDT, tag="ones")
        nc.scalar.copy(ones_sb[HD:HD + 1, :], ones_f32[HD:HD + 1, :])
        eye_sb = singles.tile([HD, P], RDT, tag="eye")
        nc.sync.dma_start(eye_sb[:], eye.bitcast(RDT))
        eye128_sb = singles.tile([P, P], RDT, tag="eye128")
        nc.sync.dma_start(eye128_sb[:], eye128.bitcast(RDT))

        qt_sb = [qt_pool.tile([P, S], RDT, tag="qt", name=f"qt{c}")
                 for c in range(QC)]
        kt_sb = kt_pool.tile([P, S], RDT, tag="kt")
        vaug_sb = [vaug_pool.tile([P, 2 * (HD + 1)], RDT, tag="vaug",
                                  name=f"vaug{k}") for k in range(DC)]

        # ============ phases 1+2: projections + RoPE ============
        with tc.tile_pool(name="rope_c", bufs=1) as ropec:
            c4_sb = ropec.tile([P, S], F32, tag="c4")
            nc.sync.dma_start(c4_sb[:], c4)
            s4_sb = ropec.tile([P, S], F32, tag="s4")
            nc.sync.dma_start(s4_sb[:], s4)

            with tc.tile_pool(name="wqp", bufs=DC) as wq_pool, \
                 tc.tile_pool(name="wkvp", bufs=DC) as wkv_pool, \
                 tc.tile_pool(name="xtp", bufs=DC + 2) as xt_pool, \
                 tc.tile_pool(name="ropet", bufs=2) as rope_pool, \
                 tc.tile_pool(name="psq", bufs=QC, space="PSUM") as psq, \
                 tc.tile_pool(name="psk", bufs=1, space="PSUM") as psk, \
                 tc.tile_pool(name="psv", bufs=1, space="PSUM") as psv, \
                 tc.tile_pool(name="pst", bufs=2, space="PSUM") as pst:
                wq_sb = [None] * DC
                wkv_sb = [None] * DC

                def load_w(d):
                    t = wq_pool.tile([P, QD], RDT, tag="wq", name=f"wq{d}")
                    nc.sync.dma_start(t[:],
                                      wq[d * P:(d + 1) * P, :].bitcast(RDT))
                    wq_sb[d] = t
                    t2 = wkv_pool.tile([P, KD + KD], RDT, tag="wkv",
                                       name=f"wkv{d}")
                    nc.sync.dma_start(t2[:],
                                      wkv[d * P:(d + 1) * P, :].bitcast(RDT))
                    wkv_sb[d] = t2

                for s in range(NB):
                    pq = [psq.tile([P, SB], F32, tag="pq", name=f"pq{c}")
                          for c in range(QC)]
                    pk = psk.tile([P, SB], F32, tag="pk")
                    xts = []
                    for d in range(DC):
                        xt = xt_pool.tile([P, SB], RDT, tag="xt",
                                          name=f"xt{d}")
                        nc.sync.dma_start(xt[:], xT[d * P:(d + 1) * P,
                                                    s * SB:(s + 1) * SB]
                                          .bitcast(RDT))
                        if s == 0:
                            load_w(d)
                        xts.append(xt)
                        st = (d == 0)
                        sp = (d == DC - 1)
                        nc.tensor.matmul(pk[:], wkv_sb[d][:, 0:KD], xt[:],
                                         start=st, stop=sp)
                        for c in range(QC):
                            nc.tensor.matmul(pq[c][:],
                                             wq_sb[d][:, c * P:(c + 1) * P],
                                             xt[:], start=st, stop=sp)
                    # evict + rope per seq-block (KT first: attention
                    # q-block 0 depends on it), overlapping PE proj work
                    sl = slice(s * SB, (s + 1) * SB)

                    def rope_block(tgt, psrc):
                        nc.scalar.copy(tgt[:, sl], psrc[:])
                        sw = rope_pool.tile([P, SB], F32, tag="sw")
                        m1 = rope_pool.tile([P, SB], F32, tag="m1")
                        hw = HD // 2
                        for b in range(0, P, hw):
                            sb2 = b + hw if (b // hw) % 2 == 0 else b - hw
                            nc.sync.dma_start(sw[b:b + hw, :],
                                              tgt[sb2:sb2 + hw, sl]
                                              .bitcast(F32))
                        nc.vector.tensor_tensor(m1[:], tgt[:, sl].bitcast(F32),
                                                c4_sb[:, sl],
                                                mybir.AluOpType.mult)
                        nc.gpsimd.tensor_tensor(sw[:], sw[:], s4_sb[:, sl],
                                                mybir.AluOpType.mult)
                        nc.vector.tensor_tensor(tgt[:, sl], m1[:], sw[:],
                                                mybir.AluOpType.add)

                    rope_block(kt_sb, pk)
                    for c in range(QC):
                        rope_block(qt_sb[c], pq[c])
                    # V: accumulate VT at N=512, then PE-transpose each
                    # 128-chunk (N=128 matmuls pay LDW per mm and 4x f32r)
                    pvt = psv.tile([P, SB], F32, tag="pv")
                    for d in range(DC):
                        nc.tensor.matmul(pvt[:], wkv_sb[d][:, KD:2 * KD],
                                         xts[d][:], start=(d == 0),
                                         stop=(d == DC - 1))
                    vts = rope_pool.tile([P, SB], RDT, tag="vts")
                    nc.scalar.copy(vts[:], pvt[:])
                    for sc in range(4):
                        trp = pst.tile([P, P], RDT, tag="trp")
                        nc.tensor.transpose(trp[:],
                                            vts[:, sc * P:(sc + 1) * P],
                                            eye128_sb[:])
                        vt = vaug_sb[4 * s + sc]
                        nc.scalar.copy(vt[:, HD:HD + 1], ones_f32[:, 0:1])
                        nc.scalar.copy(vt[:, 2 * HD + 1:2 * HD + 2],
                                       ones_f32[:, 0:1])
                        nc.scalar.copy(vt[:, 0:HD],
                                       trp[:, 0:HD].bitcast(F32))
                        nc.scalar.copy(vt[:, HD + 1:2 * HD + 1],
                                       trp[:, HD:P].bitcast(F32))
        # ================= phase 3: attention =================
        attn_sb = [attn_pool.tile([P, S], RDT, tag="attnT", name=f"attnT{c}")
                   for c in range(QC)]
        with tc.tile_pool(name="expp", bufs=6) as exp_pool, \
             tc.tile_pool(name="recipp", bufs=4) as recip_pool, \
             tc.tile_pool(name="pssc", bufs=2, space="PSUM") as pssc, \
             tc.tile_pool(name="pspv", bufs=2, space="PSUM") as pspv, \
             tc.tile_pool(name="psbc", bufs=2, space="PSUM") as psbc, \
             tc.tile_pool(name="pssh", bufs=1, space="PSUM") as pssh:
            for h in range(HQ):
                c, half = h % QC, h // QC
                kv = half
                base = half * HD
                q_rows = qt_sb[c][base:base + HD, :]
                k_rows = kt_sb[base:base + HD, :]
                for qb in range(NB):
                    nk = 4 * qb + 4
                    ppv = pspv.tile([P, SB], F32, tag="ppv")
                    for j in range(nk):
                        psc = pssc.tile([P, SB], F32, tag="psc")
                        nc.tensor.matmul(psc[:],
                                         k_rows[:, j * P:(j + 1) * P],
                                         q_rows[:, qb * SB:(qb + 1) * SB],
                                         start=True, stop=True)
                        et = exp_pool.tile([P, SB], RDT, tag="et")
                        nc.scalar.activation(et[:], psc[:],
                                             mybir.ActivationFunctionType.Exp)
                        m = j - 4 * qb
                        if m >= 0:
                            nc.vector.tensor_tensor(
                                et[:], et[:].bitcast(F32), cm_sb[m][:],
                                mybir.AluOpType.mult)
                        nc.tensor.matmul(
                            ppv[0:HD + 1, :],
                            vaug_sb[j][:, kv * (HD + 1):(kv + 1) * (HD + 1)],
                            et[:], start=(j == 0), stop=(j == nk - 1))
                    # normalize (all at base 0); sums row sits at 64
                    rc = recip_pool.tile([P, SB], RDT, tag="rc")
                    nc.vector.reciprocal(rc[HD:HD + 1, :], ppv[HD:HD + 1, :])
                    bc = psbc.tile([P, SB], F32, tag="bc")
                    nc.tensor.matmul(bc[0:HD, :], ones_sb[HD:HD + 1, :],
                                     rc[HD:HD + 1, :], start=True, stop=True)
                    # TensorTensor may read only one PSUM input: evict bc
                    # (on DVE -- ACT Copy would thrash the Exp table)
                    bcs = recip_pool.tile([P, SB], F32, tag="bcs")
                    nc.vector.tensor_copy(bcs[0:HD, :], bc[0:HD, :])
                    # PSUM-source DVE tensor_tensor measured ~8x slow:
                    # evict values via ACT, multiply SBUF x SBUF on DVE
                    av = recip_pool.tile([P, SB], F32, tag="av")
                    nc.scalar.copy(av[0:HD, :], ppv[0:HD, :])
                    if half == 0:
                        nc.vector.tensor_tensor(
                            attn_sb[c][0:HD, qb * SB:(qb + 1) * SB],
                            bcs[0:HD, :], av[0:HD, :], mybir.AluOpType.mult)
                    else:
                        stn = recip_pool.tile([P, SB], RDT, tag="stn")
                        nc.vector.tensor_tensor(
                            stn[0:HD, :], bcs[0:HD, :], av[0:HD, :],
                            mybir.AluOpType.mult)
                        # partition shift 0->64 via identity matmul on PE
                        psh = pssh.tile([P, SB], F32, tag="psh")
                        nc.tensor.matmul(psh[:], eye_sb[:],
                                         stn[0:HD, :], start=True, stop=True)
                        nc.vector.tensor_copy(
                            attn_sb[c][HD:P, qb * SB:(qb + 1) * SB],
                            psh[HD:P, :])

        if dumps is not None:
            for c in range(QC):
                nc.sync.dma_start(dumps["ad"][c * P:(c + 1) * P, :],
                                  attn_sb[c][:].bitcast(F32))

        # ================= phase 4: output projection =================
        with tc.tile_pool(name="wop", bufs=QC) as wo_pool, \
             tc.tile_pool(name="stagep", bufs=4) as stage_pool, \
             tc.tile_pool(name="pso", bufs=4, space="PSUM") as pso:
            wo_sb = []
            for c in range(QC):
                t = wo_pool.tile([P, D], RDT, tag="wo", name=f"wo{c}")
                nc.sync.dma_start(t[:], wo[c * P:(c + 1) * P, :].bitcast(RDT))
                wo_sb.append(t)
            for sc in range(S // P):
                for ob in range(NB):
                    po = pso.tile([P, SB], F32, tag="po")
                    for c in range(QC):
                        nc.tensor.matmul(po[:],
                                         attn_sb[c][:, sc * P:(sc + 1) * P],
                                         wo_sb[c][:, ob * SB:(ob + 1) * SB],
                                         start=(c == 0), stop=(c == QC - 1))
                    stg = stage_pool.tile([P, SB], F32, tag="stg")
                    nc.scalar.copy(stg[:], po[:])
                    nc.sync.dma_start(
                        outp[sc * P:(sc + 1) * P, ob * SB:(ob + 1) * SB],
                        stg[:])


_NC_CACHE = None


def _get_nc():
    global _NC_CACHE
    if _NC_CACHE is None:
        _NC_CACHE = build_kernel()
    return _NC_CACHE


def _deinterleave_cols(w):
    """Per 64-col head block: reorder cols to [evens(real), odds(imag)]."""
    d, n = w.shape
    out = np.empty_like(w)
    for h0 in range(0, n, HD):
        blk = w[:, h0:h0 + HD]
        out[:, h0:h0 + HD // 2] = blk[:, 0::2]
        out[:, h0 + HD // 2:h0 + HD] = blk[:, 1::2]
    return out


def _prep_inputs(x, wq, wk, wv, wo, freqs_cos, freqs_sin):
    scale = 1.0 / np.sqrt(HD)
    cosT = np.ascontiguousarray(freqs_cos[:S].T.astype(np.float32))  # (32,S)
    sinT = np.ascontiguousarray(freqs_sin[:S].T.astype(np.float32))
    hw = HD // 2
    c4 = np.tile(cosT, (4, 1)).astype(np.float32)              # (128, S)
    s4 = np.concatenate([-sinT, sinT, -sinT, sinT], 0).astype(np.float32)
    kk = np.arange(P, dtype=np.int64)[:, None]
    qq = np.arange(SB, dtype=np.int64)[None, :]
    cmask = np.stack([(kk <= qq - P * m).astype(np.float32) for m in range(4)])

    xTs = [np.ascontiguousarray(x[b].T) for b in range(B)]
    per_group = []
    for g in range(GROUPS):
        wq_full = np.ascontiguousarray(wq[:, g * QD:(g + 1) * QD])
        # chunk c holds heads [c, c+4] so q-head halves align with kv halves
        order = []
        for c in range(QC):
            order.extend(range(c * HD, (c + 1) * HD))
            order.extend(range((c + 4) * HD, (c + 5) * HD))
        wq_g = _deinterleave_cols(wq_full[:, order]) * scale
        wk_g = _deinterleave_cols(
            np.ascontiguousarray(wk[:, g * KD:(g + 1) * KD]))
        wv_g = np.ascontiguousarray(wv[:, g * KD:(g + 1) * KD])
        wkv_g = np.ascontiguousarray(
            np.concatenate([wk_g, wv_g], axis=1).astype(np.float32))
        wo_g = np.ascontiguousarray(wo[g * QD:(g + 1) * QD, :][order, :])
        per_group.append((wq_g.astype(np.float32), wkv_g,
                          wo_g.astype(np.float32)))

    in_maps = []
    for core in range(8):
        b, g = core // GROUPS, core % GROUPS
        wq_g, wkv_g, wo_g = per_group[g]
        in_maps.append({
            "xT": xTs[b],
            "wq": wq_g,
            "wkv": wkv_g,
            "wo": wo_g,
            "c4": c4,
            "s4": s4,
            "cmask": cmask,
            "eye": np.concatenate([np.zeros((HD, HD), np.float32),
                                   np.eye(HD, dtype=np.float32)], axis=1),
            "eye128": np.eye(P, dtype=np.float32),
        })
    return in_maps


def kernel(x, wq, wk, wv, wo, freqs_cos, freqs_sin, _trace=False):
    nc = _get_nc()
    in_maps = _prep_inputs(np.asarray(x, dtype=np.float32),
                           np.asarray(wq, dtype=np.float32),
                           np.asarray(wk, dtype=np.float32),
                           np.asarray(wv, dtype=np.float32),
                           np.asarray(wo, dtype=np.float32),
                           np.asarray(freqs_cos, dtype=np.float32),
                           np.asarray(freqs_sin, dtype=np.float32))
    res = run_bass_kernel_spmd(nc, in_maps, core_ids=list(range(8)),
                               trace=_trace)
    out = np.zeros((B, S, D), dtype=np.float32)
    for core in range(8):
        out[core // GROUPS] += res.results[core]["outp"]
    if _trace:
        kernel.last_results = res
    return out



# revision 7
# speedup vs baseline: 1.7900x; 1.7900x over previous
"""Causal GQA self-attention (B=2, S=2048, D=2048, 32 Q heads / 8 KV heads,
head_dim 64, RoPE) on 8 Trainium2 NeuronCores.

Sharding: data-parallel over batch (2) x tensor-parallel over heads (4).
Core c handles batch c//4 and head group c%4 (8 Q heads, 2 KV heads).
wq/wk/wv column-sharded, wo row-sharded; the 4 partial outputs per batch
are summed on the host at gather time (the "all-reduce").

v2: bf16 matmuls (fp32 HIGH mode streams 2 cyc/col and drew a 515us HAM
throttle), software-pipelined proj/attn/outproj phases, exp batched over
head pairs via 3D AP, causal partial ranges on diagonal tiles, row-packed
K=64 score matmuls, reciprocal_approx_fast, V projected k-major directly.

Device kernel (per core), transposed [dims, seq] layout:
  per seq-block s: KT/QT = w.T @ xT (PSUM->rope->SBUF bf16),
                   V[k,d] = lhsT=xT-chunk @ wv (k-major, into vaug+ones)
  attention per (c, qb): heads A=(c,half0) rows 0:64, B=(c,half1) 64:128
    per k-tile j: row-packed score MMs (A,B) -> psAB[128,2,512],
    one Exp ACTIVATE over both heads' valid q-range -> eAB bf16,
    causal mask mult on diagonal subranges, PV into ppvA/ppvB (ones col
    gives softmax denominators at partition 64)
  normalize: reciprocal_approx_fast -> bf16 -> PE broadcast -> mult;
    half1 rows shifted 0:64 -> 64:128 via SBUF->SBUF DMA
  outproj: po = sum_c attnT_c.T @ wo_c, evicted and DMA'd per strip
Emission interleaves proj(s) | attn(s-1) | outproj(s-2) so ACT exp work
overlaps PE projection work; PSUM = scratch2 + psAB/bc 4 + ppv 2 = 8.
"""

import sys

if "/opt/trn_rl_repo" not in sys.path:
    sys.path.insert(0, "/opt/trn_rl_repo")

import numpy as np
import ml_dtypes

import concourse.bass as bass
import concourse.tile as tile
from concourse import bacc, mybir
from concourse.bass_utils import run_bass_kernel_spmd

B = 2
S = 2048
D = 2048
N_HEAD = 32
N_KV = 8
HD = 64
GROUPS = 4
HQ = N_HEAD // GROUPS
HK = N_KV // GROUPS
QD = HQ * HD
KD = HK * HD
P = 128
SB = 512
NB = S // SB
DC = D // P
QC = QD // P

F32 = mybir.dt.float32
BF16 = mybir.dt.bfloat16
NPBF16 = ml_dtypes.bfloat16

DEBUG_DUMPS = False


def build_kernel():
    nc = bacc.Bacc("TRN2", target_bir_lowering=False, debug=False,
                   num_devices=8)

    xT = nc.dram_tensor("xT", (D, S), BF16, kind="ExternalInput").ap()
    wq = nc.dram_tensor("wq", (D, QD), BF16, kind="ExternalInput").ap()
    wkv = nc.dram_tensor("wkv", (D, KD + KD), BF16, kind="ExternalInput").ap()
    wo = nc.dram_tensor("wo", (QD, D), BF16, kind="ExternalInput").ap()
    c4 = nc.dram_tensor("c4", (P, S), BF16, kind="ExternalInput").ap()
    s4 = nc.dram_tensor("s4", (P, S), BF16, kind="ExternalInput").ap()
    cmask = nc.dram_tensor("cmask", (4, P, SB), BF16,
                           kind="ExternalInput").ap()
    outp = nc.dram_tensor("outp", (S, D), F32, kind="ExternalOutput").ap()
    dumps = None
    if DEBUG_DUMPS:
        dumps = {
            "qtd": nc.dram_tensor("qtd", (QD, S), F32,
                                  kind="ExternalOutput").ap(),
            "ktd": nc.dram_tensor("ktd", (P, S), F32,
                                  kind="ExternalOutput").ap(),
            "vd": nc.dram_tensor("vd", (DC * P, 2 * (HD + 1)), F32,
                                 kind="ExternalOutput").ap(),
            "ad": nc.dram_tensor("ad", (QD, S), F32,
                                 kind="ExternalOutput").ap(),
            "scd": nc.dram_tensor("scd", (4, P, 2, SB), F32,
                                  kind="ExternalOutput").ap(),
            "ead": nc.dram_tensor("ead", (4, P, 2, SB), F32,
                                  kind="ExternalOutput").ap(),
            "pvd": nc.dram_tensor("pvd", (P, SB), F32,
                                  kind="ExternalOutput").ap(),
        }

    with tile.TileContext(nc) as tc, \
         nc.allow_low_precision(reason="bf16 matmul pipeline"):
        _body(nc, tc, xT, wq, wkv, wo, c4, s4, cmask, outp, dumps)

    nc.compile()
    return nc


def _body(nc, tc, xT, wq, wkv, wo, c4, s4, cmask, outp, dumps=None):
    from contextlib import ExitStack

    Exp = mybir.ActivationFunctionType.Exp
    MUL = mybir.AluOpType.mult
    ADD = mybir.AluOpType.add

    dbg = {"dumps": dumps}
    ctx = ExitStack()
    with ctx:
        # ---- persistent SBUF pools ----
        qt_pool = ctx.enter_context(tc.tile_pool(name="qt", bufs=QC))
        kt_pool = ctx.enter_context(tc.tile_pool(name="kt", bufs=1))
        vaug_pool = ctx.enter_context(tc.tile_pool(name="vaug", bufs=DC))
        attn_pool = ctx.enter_context(tc.tile_pool(name="attnT", bufs=QC))
        singles = ctx.enter_context(tc.tile_pool(name="singles", bufs=1))
        w_pool = ctx.enter_context(tc.tile_pool(name="weights", bufs=1))
        xt_pool = ctx.enter_context(tc.tile_pool(name="xtp", bufs=DC + 4))
        rope_pool = ctx.enter_context(tc.tile_pool(name="ropet", bufs=2))
        exp_pool = ctx.enter_context(tc.tile_pool(name="expp", bufs=3))
        norm_pool = ctx.enter_context(tc.tile_pool(name="normp", bufs=2))
        stage_pool = ctx.enter_context(tc.tile_pool(name="stagep", bufs=3))
        # ---- PSUM pools: 2 + 4 + 1 + 1 = 8 banks ----
        scratch = ctx.enter_context(
            tc.tile_pool(name="scratch", bufs=2, space="PSUM"))
        pat = ctx.enter_context(tc.tile_pool(name="pat", bufs=2,
                                             space="PSUM"))
        ppva_pool = ctx.enter_context(
            tc.tile_pool(name="ppva", bufs=1, space="PSUM"))
        ppvb_pool = ctx.enter_context(
            tc.tile_pool(name="ppvb", bufs=1, space="PSUM"))

        # ---- constants ----
        cm_sb = []
        for m in range(4):
            t = singles.tile([P, SB], BF16, tag=f"cm{m}", name=f"cm{m}")
            nc.sync.dma_start(t[:], cmask[m])
            cm_sb.append(t)
        ones16 = singles.tile([P, HD], BF16, tag="ones16")
        nc.vector.memset(ones16[:], 1.0)
        c4_sb = singles.tile([P, S], BF16, tag="c4")
        nc.sync.dma_start(c4_sb[:], c4)
        s4_sb = singles.tile([P, S], BF16, tag="s4")
        nc.sync.dma_start(s4_sb[:], s4)

        # ---- persistent tensors ----
        qt_sb = [qt_pool.tile([P, S], BF16, tag="qt", name=f"qt{c}")
                 for c in range(QC)]
        kt_sb = kt_pool.tile([P, S], BF16, tag="kt")
        vaug_sb = [vaug_pool.tile([P, 2 * (HD + 1)], BF16, tag="vaug",
                                  name=f"vaug{k}") for k in range(DC)]
        for k in range(DC):
            nc.vector.memset(vaug_sb[k][:, HD:HD + 1], 1.0)
            nc.vector.memset(vaug_sb[k][:, 2 * HD + 1:2 * HD + 2], 1.0)
        attn_sb = [attn_pool.tile([P, S], BF16, tag="attnT",
                                  name=f"attnT{c}") for c in range(QC)]

        # ---- weights ----
        wq_sb = []
        wkv_sb = []
        for d in range(DC):
            t = w_pool.tile([P, QD], BF16, tag=f"wq{d}", name=f"wq{d}")
            nc.sync.dma_start(t[:], wq[d * P:(d + 1) * P, :])
            wq_sb.append(t)
            t2 = w_pool.tile([P, KD + KD], BF16, tag=f"wkv{d}",
                             name=f"wkv{d}")
            nc.sync.dma_start(t2[:], wkv[d * P:(d + 1) * P, :])
            wkv_sb.append(t2)
        wo_sb = []
        for c in range(QC):
            t = w_pool.tile([P, D], BF16, tag=f"wo{c}", name=f"wo{c}")
            nc.sync.dma_start(t[:], wo[c * P:(c + 1) * P, :])
            wo_sb.append(t)

        xts = [None] * DC

        def load_xt(s):
            for d in range(DC):
                t = xt_pool.tile([P, SB], BF16, tag="xt", name=f"xt{s}_{d}")
                nc.sync.dma_start(
                    t[:], xT[d * P:(d + 1) * P, s * SB:(s + 1) * SB])
                xts[d] = t

        def rope_block(tgt, psrc, sl):
            # evict PSUM -> bf16 slice of tgt, then rotate in place
            nc.scalar.copy(tgt[:, sl], psrc[:])
            sw = rope_pool.tile([P, SB], BF16, tag="sw")
            m1 = rope_pool.tile([P, SB], BF16, tag="m1")
            hw = HD // 2
            for b in range(0, P, hw):
                sb2 = b + hw if (b // hw) % 2 == 0 else b - hw
                nc.sync.dma_start(sw[b:b + hw, :], tgt[sb2:sb2 + hw, sl])
            nc.vector.tensor_tensor(m1[:], tgt[:, sl], c4_sb[:, sl], MUL)
            nc.gpsimd.tensor_tensor(sw[:], sw[:], s4_sb[:, sl], MUL)
            nc.vector.tensor_tensor(tgt[:, sl], m1[:], sw[:], ADD)

        def proj_k(s):
            sl = slice(s * SB, (s + 1) * SB)
            pk = scratch.tile([P, SB], F32, tag="scr", name=f"pk{s}")
            for d in range(DC):
                nc.tensor.matmul(pk[:], wkv_sb[d][:, 0:KD], xts[d][:],
                                 start=(d == 0), stop=(d == DC - 1))
            rope_block(kt_sb, pk, sl)

        def proj_q(s, c):
            sl = slice(s * SB, (s + 1) * SB)
            pq = scratch.tile([P, SB], F32, tag="scr", name=f"pq{s}_{c}")
            for d in range(DC):
                nc.tensor.matmul(pq[:], wq_sb[d][:, c * P:(c + 1) * P],
                                 xts[d][:], start=(d == 0),
                                 stop=(d == DC - 1))
            rope_block(qt_sb[c], pq, sl)

        def proj_v(s):
            # V[k, dv] k-major directly: lhsT = xT s-subchunk, rhs = wv
            for t4 in range(4):
                pv = scratch.tile([P, P], F32, tag="scr", name=f"pv{s}_{t4}")
                for d in range(DC):
                    nc.tensor.matmul(
                        pv[:], xts[d][:, t4 * P:(t4 + 1) * P],
                        wkv_sb[d][:, KD:2 * KD],
                        start=(d == 0), stop=(d == DC - 1))
                vt = vaug_sb[4 * s + t4]
                nc.vector.tensor_copy(vt[:, 0:HD], pv[:, 0:HD])
                nc.vector.tensor_copy(vt[:, HD + 1:2 * HD + 1],
                                      pv[:, HD:2 * HD])

        def attn_chunk(qb, c):
            # heads A=(c, half0) rows 0:64, B=(c, half1) rows 64:128
            nk = 4 * qb + 4
            qsl = slice(qb * SB, (qb + 1) * SB)
            ppvA = ppva_pool.tile([P, SB], F32, tag="ppva",
                                  name=f"ppvA{qb}_{c}")
            ppvB = ppvb_pool.tile([P, SB], F32, tag="ppvb",
                                  name=f"ppvB{qb}_{c}")
            for j in range(nk):
                m = j - 4 * qb
                qs = max(m, 0) * P  # valid q start within the block
                nq = SB - qs
                ps = pat.tile([P, 2, SB], F32, tag="pat",
                              name=f"ps{qb}_{c}_{j}")
                ea = exp_pool.tile([P, 2, SB], BF16, tag="ea",
                                   name=f"ea{qb}_{c}_{j}")
                kcols = slice(j * P, (j + 1) * P)
                qcols = slice(qb * SB + qs, (qb + 1) * SB)
                # row-packed score MMs: A on array rows 0:64, B on 64:128
                nc.tensor.matmul(ps[:, 0, qs:], kt_sb[0:HD, kcols],
                                 qt_sb[c][0:HD, qcols],
                                 start=True, stop=True)
                nc.tensor.matmul(ps[:, 1, qs:], kt_sb[HD:P, kcols],
                                 qt_sb[c][HD:P, qcols],
                                 start=True, stop=True)
                # one Exp over both heads' valid range
                nc.scalar.activation(ea[:, :, qs:], ps[:, :, qs:], Exp)
                if m >= 0:
                    nc.vector.tensor_tensor(ea[:, 0, qs:], ea[:, 0, qs:],
                                            cm_sb[m][:, qs:], MUL)
                    nc.vector.tensor_tensor(ea[:, 1, qs:], ea[:, 1, qs:],
                                            cm_sb[m][:, qs:], MUL)
                dd = dbg["dumps"]
                if dd is not None and qb == 0 and c == 0:
                    t1 = stage_pool.tile([P, 2, SB], F32, tag="dbgsc",
                                         bufs=1, name=f"dsc{j}")
                    nc.vector.tensor_copy(t1[:, :, qs:], ps[:, :, qs:])
                    nc.sync.dma_start(dd["scd"][j], t1[:])
                    t2 = stage_pool.tile([P, 2, SB], F32, tag="dbgsc",
                                         bufs=1, name=f"dea{j}")
                    nc.vector.tensor_copy(t2[:, :, qs:], ea[:, :, qs:])
                    nc.sync.dma_start(dd["ead"][j], t2[:])
                nc.tensor.matmul(ppvA[0:HD + 1, qs:],
                                 vaug_sb[j][:, 0:HD + 1], ea[:, 0, qs:],
                                 start=(j == 0), stop=(j == nk - 1))
                nc.tensor.matmul(ppvB[0:HD + 1, qs:],
                                 vaug_sb[j][:, HD + 1:2 * (HD + 1)],
                                 ea[:, 1, qs:],
                                 start=(j == 0), stop=(j == nk - 1))
            dd = dbg["dumps"]
            if dd is not None and qb == 0 and c == 0:
                t3 = stage_pool.tile([P, SB], F32, tag="dbgsc", bufs=1,
                                     name="dpv")
                nc.vector.tensor_copy(t3[:], ppvA[:])
                nc.sync.dma_start(dd["pvd"][:], t3[:])
            # ---- normalize ----
            for half, ppv in ((0, ppvA), (1, ppvB)):
                # sums sit at partition 64; reciprocal_approx_fast only
                # works at base partition 0 -> relocate via SBUF DMA
                ssum = norm_pool.tile([P, SB], F32, tag="ssum")
                nc.vector.tensor_copy(ssum[HD:HD + 1, :], ppv[HD:HD + 1, :])
                ssum0 = norm_pool.tile([P, SB], F32, tag="ssum0")
                nc.sync.dma_start(ssum0[0:1, :], ssum[HD:HD + 1, :])
                rc = norm_pool.tile([P, SB], F32, tag="rc")
                nc.vector.reciprocal_approx_fast(rc[0:1, :], ssum0[0:1, :])
                rc16 = norm_pool.tile([P, SB], BF16, tag="rc16")
                nc.vector.tensor_copy(rc16[0:1, :], rc[0:1, :])
                # PE broadcast of 1/sum to 64 partitions (bank from pat pool)
                bc = pat.tile([P, 2, SB], F32, tag="pat",
                              name=f"bc{qb}_{c}_{half}")
                nc.tensor.matmul(bc[0:HD, 0, :], ones16[0:1, :],
                                 rc16[0:1, :], start=True, stop=True)
                bcs = norm_pool.tile([P, SB], BF16, tag="bcs")
                nc.vector.tensor_copy(bcs[0:HD, :], bc[0:HD, 0, :])
                av = norm_pool.tile([P, SB], BF16, tag="av")
                nc.scalar.copy(av[0:HD, :], ppv[0:HD, :])
                if half == 0:
                    nc.vector.tensor_tensor(attn_sb[c][0:HD, qsl],
                                            bcs[0:HD, :], av[0:HD, :], MUL)
                else:
                    stn = norm_pool.tile([P, SB], BF16, tag="stn")
                    nc.vector.tensor_tensor(stn[0:HD, :], bcs[0:HD, :],
                                            av[0:HD, :], MUL)
                    # partition shift 0:64 -> 64:128 via SBUF-to-SBUF DMA
                    nc.sync.dma_start(attn_sb[c][HD:P, qsl], stn[0:HD, :])

        def outproj_unit(sc, ob):
            po = scratch.tile([P, SB], F32, tag="scr", name=f"po{sc}_{ob}")
            for c in range(QC):
                nc.tensor.matmul(po[:],
                                 attn_sb[c][:, sc * P:(sc + 1) * P],
                                 wo_sb[c][:, ob * SB:(ob + 1) * SB],
                                 start=(c == 0), stop=(c == QC - 1))
            stg = stage_pool.tile([P, SB], F32, tag="stg")
            if (sc + ob) % 2 == 0:
                nc.scalar.copy(stg[:], po[:])
            else:
                nc.vector.tensor_copy(stg[:], po[:])
            nc.sync.dma_start(
                outp[sc * P:(sc + 1) * P, ob * SB:(ob + 1) * SB], stg[:])

        # ================= pipelined emission =================
        for s in range(NB):
            load_xt(s)
            proj_k(s)
            for c in range(QC):
                proj_q(s, c)
                if s >= 1:
                    attn_chunk(s - 1, c)
                if s >= 2:
                    for ob in range(4):
                        outproj_unit(4 * (s - 2) + c, ob)
            proj_v(s)
        for c in range(QC):
            attn_chunk(NB - 1, c)
            for ob in range(4):
                outproj_unit(4 * (NB - 2) + c, ob)
        for c in range(QC):
            for ob in range(4):
                outproj_unit(4 * (NB - 1) + c, ob)

        if dumps is not None:
            for c in range(QC):
                dq = stage_pool.tile([P, S], F32, tag="dump", bufs=1,
                                     name=f"dq{c}")
                nc.vector.tensor_copy(dq[:], qt_sb[c][:])
                nc.sync.dma_start(dumps["qtd"][c * P:(c + 1) * P, :], dq[:])
                da = stage_pool.tile([P, S], F32, tag="dump", bufs=1,
                                     name=f"da{c}")
                nc.vector.tensor_copy(da[:], attn_sb[c][:])
                nc.sync.dma_start(dumps["ad"][c * P:(c + 1) * P, :], da[:])
            dk = stage_pool.tile([P, S], F32, tag="dump", bufs=1,
                                 name="dk")
            nc.vector.tensor_copy(dk[:], kt_sb[:])
            nc.sync.dma_start(dumps["ktd"][:], dk[:])
            for k in range(DC):
                dv = stage_pool.tile([P, 2 * (HD + 1)], F32, tag="dump",
                                     bufs=1, name=f"dv{k}")
                nc.vector.tensor_copy(dv[:], vaug_sb[k][:])
                nc.sync.dma_start(dumps["vd"][k * P:(k + 1) * P, :], dv[:])


_NC_CACHE = None


def _get_nc():
    global _NC_CACHE
    if _NC_CACHE is None:
        _NC_CACHE = build_kernel()
    return _NC_CACHE


def _deinterleave_cols(w):
    """Per 64-col head block: reorder cols to [evens(real), odds(imag)]."""
    d, n = w.shape
    out = np.empty_like(w)
    for h0 in range(0, n, HD):
        blk = w[:, h0:h0 + HD]
        out[:, h0:h0 + HD // 2] = blk[:, 0::2]
        out[:, h0 + HD // 2:h0 + HD] = blk[:, 1::2]
    return out


def _prep_inputs(x, wq, wk, wv, wo, freqs_cos, freqs_sin):
    scale = 1.0 / np.sqrt(HD)
    cosT = np.ascontiguousarray(freqs_cos[:S].T.astype(np.float32))  # (32,S)
    sinT = np.ascontiguousarray(freqs_sin[:S].T.astype(np.float32))
    c4 = np.tile(cosT, (4, 1)).astype(NPBF16)                  # (128, S)
    s4 = np.concatenate([-sinT, sinT, -sinT, sinT], 0).astype(NPBF16)
    kk = np.arange(P, dtype=np.int64)[:, None]
    qq = np.arange(SB, dtype=np.int64)[None, :]
    cmask = np.stack(
        [(kk <= qq - P * m).astype(np.float32) for m in range(4)]
    ).astype(NPBF16)

    xTs = [np.ascontiguousarray(x[b].T).astype(NPBF16) for b in range(B)]
    per_group = []
    for g in range(GROUPS):
        wq_full = np.ascontiguousarray(wq[:, g * QD:(g + 1) * QD])
        # chunk c holds heads [c, c+4] so q-head halves align with kv halves
        order = []
        for c in range(QC):
            order.extend(range(c * HD, (c + 1) * HD))
            order.extend(range((c + 4) * HD, (c + 5) * HD))
        wq_g = _deinterleave_cols(wq_full[:, order]) * scale
        wk_g = _deinterleave_cols(
            np.ascontiguousarray(wk[:, g * KD:(g + 1) * KD]))
        wv_g = np.ascontiguousarray(wv[:, g * KD:(g + 1) * KD])
        wkv_g = np.ascontiguousarray(
            np.concatenate([wk_g, wv_g], axis=1)).astype(NPBF16)
        wo_g = np.ascontiguousarray(
            wo[g * QD:(g + 1) * QD, :][order, :]).astype(NPBF16)
        per_group.append((wq_g.astype(NPBF16), wkv_g, wo_g))

    in_maps = []
    for core in range(8):
        b, g = core // GROUPS, core % GROUPS
        wq_g, wkv_g, wo_g = per_group[g]
        in_maps.append({
            "xT": xTs[b],
            "wq": wq_g,
            "wkv": wkv_g,
            "wo": wo_g,
            "c4": c4,
            "s4": s4,
            "cmask": cmask,
        })
    return in_maps


def kernel(x, wq, wk, wv, wo, freqs_cos, freqs_sin, _trace=False):
    nc = _get_nc()
    in_maps = _prep_inputs(np.asarray(x, dtype=np.float32),
                           np.asarray(wq, dtype=np.float32),
                           np.asarray(wk, dtype=np.float32),
                           np.asarray(wv, dtype=np.float32),
                           np.asarray(wo, dtype=np.float32),
                           np.asarray(freqs_cos, dtype=np.float32),
                           np.asarray(freqs_sin, dtype=np.float32))
    res = run_bass_kernel_spmd(nc, in_maps, core_ids=list(range(8)),
                               trace=_trace)
    out = np.zeros((B, S, D), dtype=np.float32)
    for core in range(8):
        out[core // GROUPS] += res.results[core]["outp"]
    if _trace:
        kernel.last_results = res
    return out
